# revision 58
# baseline (speedup 1.0000x reference)
"""Causal single-head attention on 8 Trainium2 NeuronCores.

Problem: x[8, 4096, 512] @ W_{Q,K,V}[512, 64] -> causal softmax attention
-> out[8, 4096, 64].

Sharding: data-parallel over batch, one batch element per core (B == n_cores
== 8), QKV weights replicated. No collectives.

Per-core design (S=4096, D=512, E=64):
  - Everything stays on-chip: x is read once (8MB), q/k/v/scores never touch
    DRAM.
  - x ingest: per-tile f32 DMA loads (viewed as f32r bit-identically) ->
    PE transposes at 1.5 cycles/row (f32r x f32r identity) -> DVE evacuates
    PSUM into a bf16 xT [d_par, s_free].
  - Q/K/V projections run bf16 x bf16 (1 PE cycle/row); scores run f32r
    (psum evacuations produce f32r); PV runs bf16 (exp writes bf16 P, V is
    stored bf16 with an appended ones column).
  - Transposed score layout ST[k_par, q_free] so the softmax denominator
    falls out of the PV matmul via the ones-column on V (row 64 of out.T
    accumulates sum_k P[k,q]); the O(S^2) inner loop needs no transposes.
  - Scores matmuls contract over E=64 only, so two k-tiles are packed into
    the PE array quadrants (tile_position (0,0)/(64,0)).
  - Causality: strictly-upper tile pairs are skipped, diagonal-crossing
    tiles get column-restricted matmuls/exp plus a GpSimd affine_select
    zeroing the 128x128 triangle of exp'd scores; softmax exp(s/8) is
    unnormalized (no max subtraction; |s|/8 <= ~6 for these inputs).
  - Software pipelining: chunk c+1's ingest/projection ops are interleaved
    between chunk c's attention pairs (between the exp and the PV matmuls,
    where the PE would otherwise stall on the Activation engine), and the
    x loads run two chunks ahead. The reps loop is flattened into the same
    pipeline so the benchmark steady state carries no per-rep bubble.
"""

import sys

sys.path.insert(0, "/opt/trn_rl_repo")
sys.path.insert(0, "/root/.axon_site/_ro/trn_rl_repo")

import numpy as np

B, S, D, E = 8, 4096, 512, 64
N_CORES = 8

_cache = {}


def _build(S=S, reps=1):
    import concourse.bass as bass
    import concourse.mybir as mybir
    import concourse.tile as tile
    from concourse import bacc
    from concourse.masks import make_identity

    F32 = mybir.dt.float32
    F32R = mybir.dt.float32r
    BF16 = mybir.dt.bfloat16
    EXP = mybir.ActivationFunctionType.Exp

    T = S // 128   # 128-row seq tiles
    C = S // 512   # 512-col q chunks
    DC = D // 128  # contraction chunks

    nc = bacc.Bacc("TRN2", target_bir_lowering=False, debug=False,
                   num_devices=N_CORES, dynamic_dma_scratch_size=65536)
    x = nc.dram_tensor("x", [S, D], F32, kind="ExternalInput").ap()
    wq = nc.dram_tensor("W_Q", [D, E], F32, kind="ExternalInput").ap()
    wk = nc.dram_tensor("W_K", [D, E], F32, kind="ExternalInput").ap()
    wv = nc.dram_tensor("W_V", [D, E], F32, kind="ExternalInput").ap()
    out = nc.dram_tensor("out", [S, E], F32, kind="ExternalOutput").ap()

    with tile.TileContext(nc) as tc:
        from contextlib import ExitStack

        with ExitStack() as ctx:
            const = ctx.enter_context(tc.tile_pool(name="const", bufs=1))
            big = ctx.enter_context(tc.tile_pool(name="big", bufs=1))
            xf32 = ctx.enter_context(tc.tile_pool(name="xf32", bufs=12))
            xin = ctx.enter_context(tc.tile_pool(name="xin", bufs=5))
            sbw = ctx.enter_context(tc.tile_pool(name="work", bufs=3))
            ptp = ctx.enter_context(tc.tile_pool(name="pt", bufs=4))
            ps12 = ctx.enter_context(tc.tile_pool(name="ps12", bufs=2, space="PSUM"))
            psst = ctx.enter_context(tc.tile_pool(name="psst", bufs=2, space="PSUM"))
            pso = ctx.enter_context(tc.tile_pool(name="pso", bufs=2, space="PSUM"))

            # ---------------- constants / big residents ----------------
            wstage = const.tile([128, DC, 2 * E], F32)
            wbf = const.tile([128, DC, 2 * E], BF16)
            wvstage = const.tile([128, DC, E], F32)
            wvbf = const.tile([128, DC, E], BF16)
            ident = const.tile([128, 128], F32)
            identr_t = const.tile([128, 128], F32R)
            ones_st = const.tile([128, T], F32)

            # x transposed: [p, t, c, s] = x[128t+s, 128c+p]; per-tile [:, t]
            # contiguous (the XBAR transpose DMA corrupts strided outputs)
            xT = big.tile([128, T, DC, 128], BF16)
            qkALL = big.tile([128, S], F32R)         # [0:64]=kT, [64:128]=qT
            QLK = big.tile([128, S], F32R)           # [0:64]=qT ; [64:128, 0:S//2]=kT odd tiles
            v_aug = big.tile([128, T, E + 1], BF16)  # v rows + ones col

            identr = identr_t[:]

            def op_consts():
                nc.sync.dma_start(wstage[:, :, 0:E],
                                  wk.rearrange("(c p) e -> p c e", p=128))
                nc.sync.dma_start(wstage[:, :, E:2 * E],
                                  wq.rearrange("(c p) e -> p c e", p=128))
                nc.sync.dma_start(wvstage[:], wv.rearrange("(c p) e -> p c e", p=128))
                nc.vector.tensor_copy(wbf[:], wstage[:])
                nc.vector.tensor_copy(wvbf[:], wvstage[:])
                make_identity(nc, ident[:])
                # the f32r copy is the official "rounded producer" walrus
                # demands for f32r-matmul operands
                nc.vector.tensor_copy(identr_t[:], ident[:])
                nc.gpsimd.memset(ones_st[:], 1.0)
                nc.vector.tensor_copy(v_aug[:, :, E:E + 1], ones_st[:])

            # ---------------- op-group constructors ----------------
            xsts = {}     # c -> loaded/cast x tiles awaiting transpose
            pso_map = {}  # c -> accumulating attention psum
            RAMP = 3      # first global chunks use the low-latency PE path

            def op_load(c):
                def f():
                    tiles = []
                    for t in range(4 * c, 4 * c + 4):
                        # f32r tile fed by a bit-identical DMA so walrus sees
                        # an f32r producer for the transpose matmuls
                        xst = xf32.tile([128, D], F32R, tag="xf", name="xst")
                        nc.sync.dma_start(
                            xst[:], x[128 * t:128 * (t + 1), :].bitcast(F32R))
                        tiles.append(xst)
                    xsts[("f", c)] = tiles
                return f

            def op_tr(c, m):
                def f():
                    xst = xsts[("f", c)][m]
                    ps_x = ps12.tile([128, D], F32R, tag="ps12", name="ps_x")
                    for d in range(DC):
                        nc.tensor.transpose(
                            ps_x[:, 128 * d:128 * (d + 1)],
                            xst[:, 128 * d:128 * (d + 1)],
                            identr)
                    nc.vector.tensor_copy(
                        xT[:, 4 * c + m, :, :],
                        ps_x[:].rearrange("p (c f) -> p c f", f=128))
                    if m == 3:
                        del xsts[("f", c)]
                return f

            def op_load_cast(c):
                def f():
                    # gpsimd casting DMA loads the chunk straight to bf16.
                    # High priority so it never queues behind exp-dependent
                    # affine_selects on the GpSimd engine.
                    with tc.high_priority():
                        xbf = xin.tile([128, 4, D], BF16, tag="xbf", name="xbf")
                        nc.gpsimd.dma_start(
                            xbf[:],
                            x[512 * c:512 * (c + 1), :].rearrange(
                                "(m p) d -> p m d", p=128))
                    xsts[("b", c)] = xbf
                return f

            def op_trdma(c, m):
                def f():
                    xbf = xsts[("b", c)]
                    nc.sync.dma_start(
                        xT[:, 4 * c + m, :, :], xbf[:, m, :], transpose=True)
                    if m == 3:
                        del xsts[("b", c)]
                return f

            def op_qk(c):
                def f():
                    ps_qk = ps12.tile([128, 512], F32, tag="ps12", name="ps_qk")
                    for d in range(DC):
                        nc.tensor.matmul(
                            ps_qk[:], wbf[:, d, :],
                            xT[:, 4 * c:4 * c + 4, d, :],
                            start=(d == 0), stop=(d == DC - 1))
                    nc.vector.tensor_copy(qkALL[:, 512 * c:512 * (c + 1)], ps_qk[:])
                return f

            def op_dup(c):
                def f():
                    # duplicates across partition halves (SBUF->SBUF DMA).
                    # High priority: these gate the next chunk's scores.
                    with tc.high_priority():
                        nc.gpsimd.dma_start(
                            QLK[0:64, 512 * c:512 * (c + 1)],
                            qkALL[64:128, 512 * c:512 * (c + 1)])
                        odd_src = qkALL[0:64, 512 * c:512 * (c + 1)].rearrange(
                            "p (a b f) -> p a b f", b=2, f=128)[:, :, 1, :]
                        nc.gpsimd.dma_start(
                            QLK[64:128, 256 * c:256 * (c + 1)].rearrange(
                                "p (a f) -> p a f", f=128),
                            odd_src)
                return f

            def op_v(c):
                def f():
                    ps_vt = ps12.tile([64, 512], F32, tag="ps12", name="ps_vt")
                    for d in range(DC):
                        nc.tensor.matmul(
                            ps_vt[:], wvbf[:, d, :],
                            xT[:, 4 * c:4 * c + 4, d, :],
                            start=(d == 0), stop=(d == DC - 1))
                    vt_sb = sbw.tile([64, 512], F32, tag="vt", name="vt_sb")
                    nc.vector.tensor_copy(vt_sb[:], ps_vt[:])
                    ps_vtr = ps12.tile([128, 4 * E], F32, tag="ps12", name="ps_vtr")
                    for m in range(4):
                        nc.tensor.transpose(
                            ps_vtr[:, E * m:E * (m + 1)],
                            vt_sb[:, 128 * m:128 * (m + 1)],
                            ident[0:64, 0:64])
                    nc.vector.tensor_copy(
                        v_aug[:, 4 * c:4 * c + 4, 0:E],
                        ps_vtr[:].rearrange("p (m e) -> p m e", e=E))
                return f

            def load_ops(gi, c):
                return [op_load(c) if gi < RAMP else op_load_cast(c)]

            def comp_ops(gi, c):
                tr = op_tr if gi < RAMP else op_trdma
                return ([tr(c, m) for m in range(4)]
                        + [op_qk(c), op_dup(c), op_v(c)])

            def op_epi(c):
                def f():
                    ps_o = pso_map.pop(c)
                    ot_sb = sbw.tile([E + 1, 512], F32, tag="ot", name="ot_sb")
                    nc.vector.tensor_copy(ot_sb[:], ps_o[:])
                    # f32 (not f32r): walrus's s3d3_mm_fp32r_restrictions
                    # rejects the 65-partition f32r transpose
                    ps_tr = ps12.tile([128, 4 * (E + 1)], F32, tag="ps12",
                                      name="ps_tr")
                    for m in range(4):
                        nc.tensor.transpose(
                            ps_tr[:, (E + 1) * m:(E + 1) * (m + 1)],
                            ot_sb[:, 128 * m:128 * (m + 1)],
                            ident[0:E + 1, 0:E + 1])
                    rec = sbw.tile([128, 4], F32, tag="rec", name="rec")
                    nc.vector.reciprocal(
                        rec[:],
                        ps_tr[:].rearrange("p (m e) -> p m e", e=E + 1)[:, :, E:E + 1])
                    out_sb = sbw.tile([128, 4, E], F32, tag="osb", name="out_sb")
                    for m in range(4):
                        nc.vector.tensor_scalar_mul(
                            out_sb[:, m, :],
                            ps_tr[:, (E + 1) * m:(E + 1) * m + E],
                            rec[:, m:m + 1])
                    nc.gpsimd.dma_start(
                        out[512 * c:512 * (c + 1), :].rearrange(
                            "(m p) e -> p m e", p=128),
                        out_sb[:])
                return f

            def emit_pair(c, j, npair, fill):
                t0, t1 = 2 * j, 2 * j + 1
                d0 = 128 * t0 - 512 * c
                d1 = d0 + 128
                c0, c1 = max(d0, 0), max(d1, 0)
                ps_pair = psst.tile([128, 1024], F32, tag="st", name="ps_pair")
                nc.tensor.matmul(
                    ps_pair[:, c0:512],
                    qkALL[0:64, 128 * t0:128 * (t0 + 1)],
                    QLK[0:64, 512 * c + c0:512 * (c + 1)],
                    start=True, stop=True, tile_position=(0, 0))
                nc.tensor.matmul(
                    ps_pair[:, 512 + c1:1024],
                    QLK[64:128, 128 * j:128 * (j + 1)],
                    qkALL[64:128, 512 * c + c1:512 * (c + 1)],
                    start=True, stop=True, tile_position=(64, 0))
                pt = ptp.tile([128, 1024], BF16, tag="pt", name="pt")
                if c1 == 0:
                    nc.scalar.activation(pt[:, 0:1024], ps_pair[:, 0:1024], EXP,
                                         scale=0.125)
                else:
                    nc.scalar.activation(pt[:, c0:512], ps_pair[:, c0:512], EXP,
                                         scale=0.125)
                    nc.scalar.activation(pt[:, 512 + c1:1024],
                                         ps_pair[:, 512 + c1:1024], EXP,
                                         scale=0.125)
                # zero the masked triangle of the diagonal tiles on GpSimd
                # (keep where q_local >= k_local)
                if 0 <= d0:
                    nc.gpsimd.affine_select(
                        out=pt[:, d0:d0 + 128],
                        in_=pt[:, d0:d0 + 128],
                        compare_op=mybir.AluOpType.is_ge, fill=0.0,
                        base=0, pattern=[[1, 128]], channel_multiplier=-1)
                if 0 <= d1 < 512:
                    nc.gpsimd.affine_select(
                        out=pt[:, 512 + d1:512 + d1 + 128],
                        in_=pt[:, 512 + d1:512 + d1 + 128],
                        compare_op=mybir.AluOpType.is_ge, fill=0.0,
                        base=0, pattern=[[1, 128]], channel_multiplier=-1)
                # next-chunk ingest/projection work lands here, where the PE
                # would otherwise stall waiting for the exp above
                for f in fill:
                    f()
                if j == 0:
                    ps_o = pso.tile([E + 1, 512], F32, tag="pso", name="ps_o")
                    pso_map[c] = ps_o
                else:
                    ps_o = pso_map[c]
                nc.tensor.matmul(
                    ps_o[:, c0:512], v_aug[:, t0, :], pt[:, c0:512],
                    start=(j == 0), stop=False)
                nc.tensor.matmul(
                    ps_o[:, c1:512], v_aug[:, t1, :], pt[:, 512 + c1:1024],
                    start=False, stop=(j == npair - 1))

            # ---------------- pipelined emission ----------------
            # x loads run 2 chunks ahead; transposes + projections run 1
            # chunk ahead, interleaved between the attention pairs.
            seq = [c for _r in range(reps) for c in range(C)]
            for i in range(min(3, len(seq))):
                for f in load_ops(i, seq[i]):
                    f()
            op_consts()
            for f in comp_ops(0, seq[0]):
                f()
            pend = []  # epilogue of previous chunk, deferred into this one
            for i, c in enumerate(seq):
                comps = comp_ops(i + 1, seq[i + 1]) if i + 1 < len(seq) else []
                loads = load_ops(i + 3, seq[i + 3]) if i + 3 < len(seq) else []
                fills = list(pend) + loads
                pend = []
                fills += comps
                npair = 2 * c + 2
                lo = 0
                for j in range(npair):
                    hi = ((j + 1) * len(fills)) // npair
                    emit_pair(c, j, npair, fills[lo:hi])
                    lo = hi
                pend = [op_epi(c)]
            for f in pend:
                f()

    nc.compile()
    return nc


def _get_nc():
    if "nc" not in _cache:
        _cache["nc"] = _build()
    return _cache["nc"]


def kernel(x, W_Q, W_K, W_V):
    from concourse import bass_utils

    x = np.ascontiguousarray(np.asarray(x, dtype=np.float32))
    W_Q = np.ascontiguousarray(np.asarray(W_Q, dtype=np.float32))
    W_K = np.ascontiguousarray(np.asarray(W_K, dtype=np.float32))
    W_V = np.ascontiguousarray(np.asarray(W_V, dtype=np.float32))
    nc = _get_nc()
    in_maps = [
        {"x": x[b], "W_Q": W_Q, "W_K": W_K, "W_V": W_V} for b in range(B)
    ]
    res = bass_utils.run_bass_kernel_spmd(nc, in_maps, core_ids=list(range(N_CORES)))
    return np.stack([res.results[b]["out"] for b in range(B)], axis=0)


# revision 59
# speedup vs baseline: 1.3452x; 1.3452x over previous
"""Causal single-head attention on 8 Trainium2 NeuronCores.

Problem: x[8, 4096, 512] @ W_{Q,K,V}[512, 64] -> causal softmax attention
-> out[8, 4096, 64].

Sharding: data-parallel over batch, one batch element per core (B == n_cores
== 8), QKV weights replicated. No collectives.

Per-core design (S=4096, D=512, E=64):
  - Everything stays on-chip: x is read once (8MB), q/k/v/scores never touch
    DRAM.
  - x ingest: per-tile f32 DMA loads (viewed as f32r bit-identically) ->
    PE transposes at 1.5 cycles/row (f32r x f32r identity) -> DVE evacuates
    PSUM into a bf16 xT [d_par, s_free].
  - Q/K/V projections run bf16 x bf16 (1 PE cycle/row); scores run f32r
    (psum evacuations produce f32r); PV runs bf16 (exp writes bf16 P, V is
    stored bf16 with an appended ones column).
  - Transposed score layout ST[k_par, q_free] so the softmax denominator
    falls out of the PV matmul via the ones-column on V (row 64 of out.T
    accumulates sum_k P[k,q]); the O(S^2) inner loop needs no transposes.
  - Scores matmuls contract over E=64 only, so two k-tiles are packed into
    the PE array quadrants (tile_position (0,0)/(64,0)).
  - Causality: strictly-upper tile pairs are skipped, diagonal-crossing
    tiles get column-restricted matmuls/exp plus a GpSimd affine_select
    zeroing the 128x128 triangle of exp'd scores; softmax exp(s/8) is
    unnormalized (no max subtraction; |s|/8 <= ~6 for these inputs).
  - Software pipelining: chunk c+1's ingest/projection ops are interleaved
    between chunk c's attention pairs (between the exp and the PV matmuls,
    where the PE would otherwise stall on the Activation engine), and the
    x loads run two chunks ahead. The reps loop is flattened into the same
    pipeline so the benchmark steady state carries no per-rep bubble.
"""

import sys

sys.path.insert(0, "/opt/trn_rl_repo")
sys.path.insert(0, "/root/.axon_site/_ro/trn_rl_repo")

import numpy as np

B, S, D, E = 8, 4096, 512, 64
N_CORES = 8

_cache = {}


def _build(S=S, reps=1):
    import concourse.bass as bass
    import concourse.mybir as mybir
    import concourse.tile as tile
    from concourse import bacc
    from concourse.masks import make_identity

    F32 = mybir.dt.float32
    F32R = mybir.dt.float32r
    BF16 = mybir.dt.bfloat16
    EXP = mybir.ActivationFunctionType.Exp

    T = S // 128   # 128-row seq tiles
    C = S // 512   # 512-col q chunks
    DC = D // 128  # contraction chunks

    nc = bacc.Bacc("TRN2", target_bir_lowering=False, debug=False,
                   num_devices=N_CORES)
    x = nc.dram_tensor("x", [S, D], F32, kind="ExternalInput").ap()
    wq = nc.dram_tensor("W_Q", [D, E], F32, kind="ExternalInput").ap()
    wk = nc.dram_tensor("W_K", [D, E], F32, kind="ExternalInput").ap()
    wv = nc.dram_tensor("W_V", [D, E], F32, kind="ExternalInput").ap()
    out = nc.dram_tensor("out", [S, E], F32, kind="ExternalOutput").ap()

    with tile.TileContext(nc) as tc:
        from contextlib import ExitStack

        with ExitStack() as ctx:
            const = ctx.enter_context(tc.tile_pool(name="const", bufs=1))
            big = ctx.enter_context(tc.tile_pool(name="big", bufs=1))
            xf32 = ctx.enter_context(tc.tile_pool(name="xf32", bufs=10))
            sbw = ctx.enter_context(tc.tile_pool(name="work", bufs=3))
            ptp = ctx.enter_context(tc.tile_pool(name="pt", bufs=4))
            ps12 = ctx.enter_context(tc.tile_pool(name="ps12", bufs=2, space="PSUM"))
            psst = ctx.enter_context(tc.tile_pool(name="psst", bufs=2, space="PSUM"))
            pso = ctx.enter_context(tc.tile_pool(name="pso", bufs=2, space="PSUM"))

            # ---------------- constants / big residents ----------------
            wstage = const.tile([128, DC, 2 * E], F32)
            wbf = const.tile([128, DC, 2 * E], BF16)
            wvstage = const.tile([128, DC, E], F32)
            wvbf = const.tile([128, DC, E], BF16)
            ident = const.tile([128, 128], F32)
            identr_t = const.tile([128, 128], F32R)
            ones_st = const.tile([128, T], F32)

            xT = big.tile([128, DC, S], BF16)        # x transposed, d on partitions
            qkALL = big.tile([128, S], F32R)         # [0:64]=kT, [64:128]=qT
            QLK = big.tile([128, S], F32R)           # [0:64]=qT ; [64:128, 0:S//2]=kT odd tiles
            v_aug = big.tile([128, T, E + 1], BF16)  # v rows + ones col

            identr = identr_t[:]

            def op_consts():
                nc.sync.dma_start(wstage[:, :, 0:E],
                                  wk.rearrange("(c p) e -> p c e", p=128))
                nc.sync.dma_start(wstage[:, :, E:2 * E],
                                  wq.rearrange("(c p) e -> p c e", p=128))
                nc.sync.dma_start(wvstage[:], wv.rearrange("(c p) e -> p c e", p=128))
                nc.vector.tensor_copy(wbf[:], wstage[:])
                nc.vector.tensor_copy(wvbf[:], wvstage[:])
                make_identity(nc, ident[:])
                # the f32r copy is the official "rounded producer" walrus
                # demands for f32r-matmul operands
                nc.vector.tensor_copy(identr_t[:], ident[:])
                nc.gpsimd.memset(ones_st[:], 1.0)
                nc.vector.tensor_copy(v_aug[:, :, E:E + 1], ones_st[:])

            # ---------------- op-group constructors ----------------
            xsts = {}     # c -> list of loaded x tiles awaiting transpose
            pso_map = {}  # c -> accumulating attention psum

            def op_load(c):
                def f():
                    tiles = []
                    for t in range(4 * c, 4 * c + 4):
                        # f32r tile fed by a bit-identical DMA so walrus sees
                        # an f32r producer for the transpose matmuls
                        xst = xf32.tile([128, D], F32R, tag="xf", name="xst")
                        nc.sync.dma_start(
                            xst[:], x[128 * t:128 * (t + 1), :].bitcast(F32R))
                        tiles.append(xst)
                    xsts[c] = tiles
                return f

            def op_tr(c, m):
                def f():
                    xst = xsts[c][m]
                    ps_x = ps12.tile([128, D], F32R, tag="ps12", name="ps_x")
                    for d in range(DC):
                        nc.tensor.transpose(
                            ps_x[:, 128 * d:128 * (d + 1)],
                            xst[:, 128 * d:128 * (d + 1)],
                            identr)
                    nc.vector.tensor_copy(
                        xT[:, :, 128 * (4 * c + m):128 * (4 * c + m + 1)],
                        ps_x[:].rearrange("p (c f) -> p c f", f=128))
                    if m == 3:
                        del xsts[c]
                return f

            def op_qk(c):
                def f():
                    ps_qk = ps12.tile([128, 512], F32, tag="ps12", name="ps_qk")
                    for d in range(DC):
                        nc.tensor.matmul(
                            ps_qk[:], wbf[:, d, :],
                            xT[:, d, 512 * c:512 * (c + 1)],
                            start=(d == 0), stop=(d == DC - 1))
                    nc.vector.tensor_copy(qkALL[:, 512 * c:512 * (c + 1)], ps_qk[:])
                return f

            def op_dup(c):
                def f():
                    # duplicates across partition halves (SBUF->SBUF DMA).
                    # High priority: these gate the next chunk's scores.
                    with tc.high_priority():
                        nc.sync.dma_start(
                            QLK[0:64, 512 * c:512 * (c + 1)],
                            qkALL[64:128, 512 * c:512 * (c + 1)])
                        odd_src = qkALL[0:64, 512 * c:512 * (c + 1)].rearrange(
                            "p (a b f) -> p a b f", b=2, f=128)[:, :, 1, :]
                        nc.sync.dma_start(
                            QLK[64:128, 256 * c:256 * (c + 1)].rearrange(
                                "p (a f) -> p a f", f=128),
                            odd_src)
                return f

            def op_v(c):
                def f():
                    ps_vt = ps12.tile([64, 512], F32, tag="ps12", name="ps_vt")
                    for d in range(DC):
                        nc.tensor.matmul(
                            ps_vt[:], wvbf[:, d, :],
                            xT[:, d, 512 * c:512 * (c + 1)],
                            start=(d == 0), stop=(d == DC - 1))
                    vt_sb = sbw.tile([64, 512], F32, tag="vt", name="vt_sb")
                    nc.vector.tensor_copy(vt_sb[:], ps_vt[:])
                    ps_vtr = ps12.tile([128, 4 * E], F32, tag="ps12", name="ps_vtr")
                    for m in range(4):
                        nc.tensor.transpose(
                            ps_vtr[:, E * m:E * (m + 1)],
                            vt_sb[:, 128 * m:128 * (m + 1)],
                            ident[0:64, 0:64])
                    nc.vector.tensor_copy(
                        v_aug[:, 4 * c:4 * c + 4, 0:E],
                        ps_vtr[:].rearrange("p (m e) -> p m e", e=E))
                return f

            def load_ops(c):
                return [op_load(c)]

            def comp_ops(c):
                return ([op_tr(c, m) for m in range(4)]
                        + [op_qk(c), op_dup(c), op_v(c)])

            def op_epi(c):
                def f():
                    ps_o = pso_map.pop(c)
                    ot_sb = sbw.tile([E + 1, 512], F32, tag="ot", name="ot_sb")
                    nc.vector.tensor_copy(ot_sb[:], ps_o[:])
                    # f32 (not f32r): walrus's s3d3_mm_fp32r_restrictions
                    # rejects the 65-partition f32r transpose
                    ps_tr = ps12.tile([128, 4 * (E + 1)], F32, tag="ps12",
                                      name="ps_tr")
                    for m in range(4):
                        nc.tensor.transpose(
                            ps_tr[:, (E + 1) * m:(E + 1) * (m + 1)],
                            ot_sb[:, 128 * m:128 * (m + 1)],
                            ident[0:E + 1, 0:E + 1])
                    rec = sbw.tile([128, 4], F32, tag="rec", name="rec")
                    nc.vector.reciprocal(
                        rec[:],
                        ps_tr[:].rearrange("p (m e) -> p m e", e=E + 1)[:, :, E:E + 1])
                    out_sb = sbw.tile([128, 4, E], F32, tag="osb", name="out_sb")
                    for m in range(4):
                        nc.vector.tensor_scalar_mul(
                            out_sb[:, m, :],
                            ps_tr[:, (E + 1) * m:(E + 1) * m + E],
                            rec[:, m:m + 1])
                    nc.sync.dma_start(
                        out[512 * c:512 * (c + 1), :].rearrange(
                            "(m p) e -> p m e", p=128),
                        out_sb[:])
                return f

            def emit_pair(c, j, npair, fill):
                t0, t1 = 2 * j, 2 * j + 1
                d0 = 128 * t0 - 512 * c
                d1 = d0 + 128
                c0, c1 = max(d0, 0), max(d1, 0)
                ps_pair = psst.tile([128, 1024], F32, tag="st", name="ps_pair")
                nc.tensor.matmul(
                    ps_pair[:, c0:512],
                    qkALL[0:64, 128 * t0:128 * (t0 + 1)],
                    QLK[0:64, 512 * c + c0:512 * (c + 1)],
                    start=True, stop=True, tile_position=(0, 0))
                nc.tensor.matmul(
                    ps_pair[:, 512 + c1:1024],
                    QLK[64:128, 128 * j:128 * (j + 1)],
                    qkALL[64:128, 512 * c + c1:512 * (c + 1)],
                    start=True, stop=True, tile_position=(64, 0))
                pt = ptp.tile([128, 1024], BF16, tag="pt", name="pt")
                if c1 == 0:
                    nc.scalar.activation(pt[:, 0:1024], ps_pair[:, 0:1024], EXP,
                                         scale=0.125)
                else:
                    nc.scalar.activation(pt[:, c0:512], ps_pair[:, c0:512], EXP,
                                         scale=0.125)
                    nc.scalar.activation(pt[:, 512 + c1:1024],
                                         ps_pair[:, 512 + c1:1024], EXP,
                                         scale=0.125)
                # zero the masked triangle of the diagonal tiles on GpSimd
                # (keep where q_local >= k_local)
                if 0 <= d0:
                    nc.gpsimd.affine_select(
                        out=pt[:, d0:d0 + 128],
                        in_=pt[:, d0:d0 + 128],
                        compare_op=mybir.AluOpType.is_ge, fill=0.0,
                        base=0, pattern=[[1, 128]], channel_multiplier=-1)
                if 0 <= d1 < 512:
                    nc.gpsimd.affine_select(
                        out=pt[:, 512 + d1:512 + d1 + 128],
                        in_=pt[:, 512 + d1:512 + d1 + 128],
                        compare_op=mybir.AluOpType.is_ge, fill=0.0,
                        base=0, pattern=[[1, 128]], channel_multiplier=-1)
                # next-chunk ingest/projection work lands here, where the PE
                # would otherwise stall waiting for the exp above
                for f in fill:
                    f()
                if j == 0:
                    ps_o = pso.tile([E + 1, 512], F32, tag="pso", name="ps_o")
                    pso_map[c] = ps_o
                else:
                    ps_o = pso_map[c]
                nc.tensor.matmul(
                    ps_o[:, c0:512], v_aug[:, t0, :], pt[:, c0:512],
                    start=(j == 0), stop=False)
                nc.tensor.matmul(
                    ps_o[:, c1:512], v_aug[:, t1, :], pt[:, 512 + c1:1024],
                    start=False, stop=(j == npair - 1))

            # ---------------- pipelined emission ----------------
            # x loads run 2 chunks ahead; transposes + projections run 1
            # chunk ahead, interleaved between the attention pairs.
            seq = [c for _r in range(reps) for c in range(C)]
            for i in range(min(2, len(seq))):
                for f in load_ops(seq[i]):
                    f()
            op_consts()
            for f in comp_ops(seq[0]):
                f()
            pend = []  # epilogue of previous chunk, deferred into this one
            for i, c in enumerate(seq):
                comps = comp_ops(seq[i + 1]) if i + 1 < len(seq) else []
                loads = load_ops(seq[i + 2]) if i + 2 < len(seq) else []
                fills = list(pend) + loads
                pend = []
                fills += comps
                npair = 2 * c + 2
                lo = 0
                for j in range(npair):
                    hi = ((j + 1) * len(fills)) // npair
                    emit_pair(c, j, npair, fills[lo:hi])
                    lo = hi
                pend = [op_epi(c)]
            for f in pend:
                f()

    nc.compile()
    return nc


def _get_nc():
    if "nc" not in _cache:
        _cache["nc"] = _build()
    return _cache["nc"]


def kernel(x, W_Q, W_K, W_V):
    from concourse import bass_utils

    x = np.ascontiguousarray(np.asarray(x, dtype=np.float32))
    W_Q = np.ascontiguousarray(np.asarray(W_Q, dtype=np.float32))
    W_K = np.ascontiguousarray(np.asarray(W_K, dtype=np.float32))
    W_V = np.ascontiguousarray(np.asarray(W_V, dtype=np.float32))
    nc = _get_nc()
    in_maps = [
        {"x": x[b], "W_Q": W_Q, "W_K": W_K, "W_V": W_V} for b in range(B)
    ]
    res = bass_utils.run_bass_kernel_spmd(nc, in_maps, core_ids=list(range(N_CORES)))
    return np.stack([res.results[b]["out"] for b in range(B)], axis=0)
